# revision 1
# baseline (speedup 1.0000x reference)
"""Trainium2 Bass kernel for DirectedNetworkFeatureExtractor (GAT+FC GNN), v2.

Structure (per layer, nodes sharded 8-way, transposed-state free):
  node phase: h = x @ Wg (PSUM), als/ald reductions, h -> h_bounce DRAM
  AllGather h table (bf16, 8 x 1568-row chunks, Shared out)
  edge phase per (window-group, chunk) section:
    dma_gather h[src] (256B rows, int16 idx, 4 table chunks)
    als per edge (G * a_s reduce), ald per edge via host-built one-hot S0T
    matmuls, p = exp(leaky_relu(als+ald)), segment-sum via host-built S0
    one-hot matmuls accumulating (sum p*h, sum p) per dst window in PSUM
  x2 = agg/sum_p + bias, relu; x1 = fc branch; concat -> next layer state.

v2 vs v1: S0/S0T one-hots precomputed on host (DRAM inputs, DMA-streamed),
window-impure gather tiles (fewer padded slots), AGC=1568, Shared AG out.
"""
import math
import sys

sys.path.insert(0, "/opt/trn_rl_repo")

import numpy as np
import ml_dtypes

import concourse.bass as bass
import concourse.bacc as bacc
import concourse.tile as tile
from concourse import mybir

BF = ml_dtypes.bfloat16
P = 128

N_NODES = 100_000
N_CORES = 8
HEADS = 4
NCHUNK = 4
GB = 4          # windows per gather group
AGC = 3136      # AllGather chunk rows (= CHSZ/8 so one AG = one table chunk)


# --------------------------------------------------------------------------
# host-side graph preprocessing (untimed)
# --------------------------------------------------------------------------
def prep_structure(edge_index, n_nodes, n_cores):
    src = np.asarray(edge_index[0]).astype(np.int64)
    dst = np.asarray(edge_index[1]).astype(np.int64)
    shard = n_nodes // n_cores
    W = math.ceil(shard / P)
    SH = W * P
    TAB = n_cores * SH
    CHSZ = TAB // NCHUNK
    NE = len(src)

    core = dst // shard
    dloc = dst - core * shard
    win = dloc // P
    # AG-chunk-major table layout
    agc = AGC
    assert SH % agc == 0 and CHSZ % (agc * n_cores) == 0
    loc = src % shard
    rnk = src // shard
    a0 = loc // agc
    grow = (a0 * n_cores + rnk) * agc + (loc % agc)   # table row
    chk = grow // CHSZ
    rel = (grow - chk * CHSZ).astype(np.int64)

    NG = math.ceil(W / GB)
    g_of_w = np.arange(W) // GB

    # section (g, c) sizes pooled across windows, max over cores
    cnt = np.zeros((n_cores, NG, NCHUNK), np.int64)
    np.add.at(cnt, (core, g_of_w[win], chk), 1)
    sec_tiles = (cnt.max(axis=0) + P - 1) // P        # [NG, NCHUNK]

    # global tile base per section; plan in (g, then c) order
    plan = []
    t_run = 0
    for g in range(NG):
        ws = list(range(g * GB, min((g + 1) * GB, W)))
        g_off = t_run
        secs = []
        for c in range(NCHUNK):
            nt = int(sec_tiles[g, c])
            secs.append(dict(off=t_run, ntiles=nt, mm=None))
            t_run += nt
        plan.append(dict(windows=ws, off=g_off,
                         ntiles=t_run - g_off, secs=secs))
    TT = t_run

    idx16 = np.zeros((n_cores, TT * P), np.int16)

    # per-core slot assignment: edges sorted by (group, chunk, window, dloc)
    order = np.lexsort((np.arange(NE), dloc, chk, g_of_w[win], core))
    s_core = core[order]
    s_chk = chk[order]
    s_g = g_of_w[win[order]]
    s_win = win[order]
    s_rel = rel[order]
    s_dloc = dloc[order]

    # position within each (core, g, c) run
    gid = (s_core * NG + s_g) * NCHUNK + s_chk
    first = np.r_[True, gid[1:] != gid[:-1]]
    starts = np.flatnonzero(first)
    run_id = np.cumsum(first) - 1
    pos = np.arange(NE) - starts[run_id]

    sec_base = np.zeros((NG, NCHUNK), np.int64)
    for g in range(NG):
        for c in range(NCHUNK):
            sec_base[g, c] = plan[g]["secs"][c]["off"]
    tile_i = sec_base[s_g, s_chk] + pos // P
    part_i = pos % P
    idx16[s_core, tile_i * P + part_i] = s_rel.astype(np.int16)

    # slot -> (window, local slot) per core; -1 window means pad
    slot_win = np.full((n_cores, TT * P), -1, np.int8)   # window - g*GB
    slot_d = np.zeros((n_cores, TT * P), np.int8)        # dloc % 128
    slot_win[s_core, tile_i * P + part_i] = (s_win - s_g * GB).astype(np.int8)
    slot_d[s_core, tile_i * P + part_i] = (s_dloc % P).astype(np.int8)

    # ---- MM entries per section: (tile, local window) pairs with edges.
    # Shared across cores: entry exists if ANY core has edges there; S0/S0T
    # content is per-core.  Entry order: per section, by (window, tile).
    entries = []           # list of (g, c, tile_global, wl)
    for g in range(NG):
        nw = len(plan[g]["windows"])
        for ci in range(NCHUNK):
            sec = plan[g]["secs"][ci]
            nt = sec["ntiles"]
            t0 = sec["off"]
            if nt == 0:
                sec["mm"] = []
                sec["mm0"] = len(entries)
                continue
            # which (tile, wl) pairs are nonempty on any core
            sw = slot_win[:, t0 * P:(t0 + nt) * P].reshape(n_cores, nt, P)
            mm = []
            for wl in range(nw):
                tiles = np.flatnonzero((sw == wl).any(axis=(0, 2)))
                for t in tiles:
                    mm.append((int(t), wl))
            # order by tile then window for PSUM run grouping by window:
            # group by window (runs), tiles ascending within run
            mm.sort(key=lambda x: (x[1], x[0]))
            sec["mm"] = mm
            sec["mm0"] = len(entries)
            for (t, wl) in mm:
                entries.append((g, ci, t0 + t, wl))
    NMM = len(entries)

    # host one-hots per core: S0 [NMM, P(e), P(d)], S0T transposed
    S0 = np.zeros((n_cores, NMM, P, P), BF)
    e_idx = np.arange(P)
    for mi, (g, ci, tg, wl) in enumerate(entries):
        for cr in range(n_cores):
            swv = slot_win[cr, tg * P:(tg + 1) * P]
            sdv = slot_d[cr, tg * P:(tg + 1) * P]
            m = swv == wl
            if m.any():
                S0[cr, mi, e_idx[m], sdv[m].astype(np.int64)] = 1.0
    S0T = np.ascontiguousarray(np.transpose(S0, (0, 1, 3, 2)))

    # pack idx for dma_gather: element i at [r, i//16] for r%16 == i%16
    j = np.arange(TT * 8)
    r16 = np.arange(16)
    packed = idx16[:, (j[None, :] * 16 + r16[:, None]).reshape(16, -1)]
    idx_packed = np.tile(packed, (1, 8, 1))       # [cores, 128, TT*8]

    return dict(
        shard=shard, W=W, SH=SH, TT=TT, TAB=TAB, CHSZ=CHSZ, NG=NG,
        plan=plan, NMM=NMM, agc=agc,
        idx=np.ascontiguousarray(idx_packed),
        S0=np.ascontiguousarray(S0.reshape(n_cores, NMM * P, P)),
        S0T=np.ascontiguousarray(S0T.reshape(n_cores, NMM * P, P)),
    )


def prep_weights(inputs):
    def blocks(w):
        k = w.shape[0]
        return np.ascontiguousarray(w.reshape(k // P, P, w.shape[1]).astype(BF))

    def rep_row(v):
        return np.broadcast_to(np.asarray(v, np.float32), (P, P)).copy()

    g = lambda n: np.asarray(inputs[n], np.float32)
    layers = [dict(
        gw=blocks(g("g1_W")), fw=blocks(g("fc1_W")),
        a_s=rep_row(g("g1_as").reshape(-1)).astype(BF),
        a_d=rep_row(g("g1_ad").reshape(-1)).astype(BF),
        gb=rep_row(g("g1_b")), fb=g("fc1_b").reshape(P, 1).astype(np.float32),
    )]
    for i in range(2):
        layers.append(dict(
            gw=blocks(g("mg_W")[i]), fw=blocks(g("mfc_W")[i]),
            a_s=rep_row(g("mg_as")[i].reshape(-1)).astype(BF),
            a_d=rep_row(g("mg_ad")[i].reshape(-1)).astype(BF),
            gb=rep_row(g("mg_b")[i]), fb=g("mfc_b")[i].reshape(P, 1).astype(np.float32),
        ))
    layers.append(dict(
        gw=blocks(g("fg_W")), fw=blocks(g("ffc_W")),
        a_s=rep_row(g("fg_as").reshape(-1)).astype(BF),
        a_d=rep_row(g("fg_ad").reshape(-1)).astype(BF),
        gb=rep_row(g("fg_b")), fb=rep_row(g("ffc_b")),
    ))
    return layers


# --------------------------------------------------------------------------
# device program
# --------------------------------------------------------------------------
def build_program(st, n_cores):
    SH, W, TT, TAB, CHSZ = st["SH"], st["W"], st["TT"], st["TAB"], st["CHSZ"]
    agc = st["agc"]
    plan = st["plan"]
    NMM = st["NMM"]
    dt = mybir.dt
    f32, bf16, i16, i8 = dt.float32, dt.bfloat16, dt.int16, dt.int8
    HL = [HEADS, HEADS, HEADS, 1]
    GNTMAX = max(g["ntiles"] for g in plan)
    SECMAX = max(s["ntiles"] for g in plan for s in g["secs"])
    MMMAX = max(len(s["mm"]) for g in plan for s in g["secs"])

    nc = bacc.Bacc(None)

    def inp(name, shape, d):
        return nc.declare_dram_parameter(name, list(shape), d, isOutput=False)

    x_in = inp("x", (SH, P), f32)
    idx_in = inp("idx", (P, TT * 8), i16)
    s0_in = inp("S0", (NMM * P, P), bf16)
    s0t_in = inp("S0T", (NMM * P, P), bf16)
    iota_in = inp("iota8", (P, P), i8)
    iotac_in = inp("iotac8", (P, 1), i8)
    lw = []
    for L in range(4):
        K = 1 if L == 0 else 2
        lw.append(dict(
            gw=inp(f"gw{L}", (K, P, P), bf16),
            fw=inp(f"fw{L}", (K, P, P), bf16),
            a_s=inp(f"as{L}", (P, P), bf16),
            a_d=inp(f"ad{L}", (P, P), bf16),
            gb=inp(f"gb{L}", (P, P), f32),
            fb=inp(f"fb{L}", (P, 1) if L < 3 else (P, P), f32),
        ))
    out_t = nc.declare_dram_parameter("out", [SH, P], f32, isOutput=True)

    CH = 512
    chunks = [(c, min(CH, SH - c)) for c in range(0, SH, CH)]

    with tile.TileContext(nc) as tc:
        with (
            tc.tile_pool(name="res", bufs=1) as res,
            tc.tile_pool(name="wts", bufs=1) as wts,
            tc.tile_pool(name="nwork", bufs=3) as nwork,
            tc.tile_pool(name="ework", bufs=2) as ework,
            tc.tile_pool(name="psA", bufs=2, space="PSUM") as psA,
            tc.tile_pool(name="psB", bufs=2, space="PSUM") as psB,
            tc.tile_pool(name="dram", bufs=1, space="DRAM") as dram,
        ):
            # ---------------- residents
            iota8 = res.tile([P, P], i8)
            nc.sync.dma_start(iota8[:], iota_in[:])
            iotac8 = res.tile([P, 1], i8)
            nc.sync.dma_start(iotac8[:], iotac_in[:])
            ident = res.tile([P, P], bf16)
            nc.vector.tensor_tensor(
                out=ident[:], in0=iotac8[:].to_broadcast([P, P]), in1=iota8[:],
                op=mybir.AluOpType.is_equal)

            wt = []
            for L in range(4):
                K = 1 if L == 0 else 2
                d = {}
                for nm in ("gw", "fw"):
                    t_ = wts.tile([P, K, P], bf16, tag=f"{nm}{L}")
                    nc.sync.dma_start(t_[:], lw[L][nm][:].rearrange("k p q -> p k q"))
                    d[nm] = t_
                for nm in ("a_s", "a_d"):
                    t_ = wts.tile([P, P], bf16, tag=f"{nm}{L}")
                    nc.sync.dma_start(t_[:], lw[L][nm][:])
                    d[nm] = t_
                t_ = wts.tile([P, P], f32, tag=f"gb{L}")
                nc.sync.dma_start(t_[:], lw[L]["gb"][:])
                d["gb"] = t_
                t_ = wts.tile([P, 1] if L < 3 else [P, P], f32, tag=f"fb{L}")
                nc.sync.dma_start(t_[:], lw[L]["fb"][:])
                d["fb"] = t_
                wt.append(d)

            # ---------------- DRAM scratch
            sA = [dram.tile([P, SH], bf16, tag=f"sA{i}", name=f"sA{i}") for i in range(3)]
            sB = [dram.tile([P, SH], bf16, tag=f"sB{i}", name=f"sB{i}") for i in range(3)]
            h_bounce = dram.tile([SH, P], bf16, tag="hb")
            tabs = [[dram.tile([CHSZ, P], bf16, tag=f"tab{L}_{ci}",
                               name=f"tab{L}_{ci}", addr_space="Shared")
                     for ci in range(NCHUNK)] for L in range(4)]
            x4_dram = dram.tile([SH, P], bf16, tag="x4")

            # ---------------- x -> transposed state
            for i in range(W):
                xt = nwork.tile([P, P], f32, tag="xin")
                nc.sync.dma_start(xt[:], x_in[i * P:(i + 1) * P, :])
                xb = nwork.tile([P, P], bf16, tag="xbf")
                nc.vector.tensor_copy(out=xb[:], in_=xt[:])
                tp = psB.tile([P, P], bf16, tag="tp", bufs=1)
                nc.tensor.transpose(out=tp[:], in_=xb[:], identity=ident[:])
                xTb = nwork.tile([P, P], bf16, tag="xT")
                nc.vector.tensor_copy(out=xTb[:], in_=tp[:])
                nc.sync.dma_start(sA[0][:, i * P:(i + 1) * P], xTb[:])

            # ---------------- layers
            for L in range(4):
                K = 1 if L == 0 else 2
                H = HL[L]
                C = P // H
                w = wt[L]
                if L == 0:
                    in_blk = [sA[0]]
                elif L == 1:
                    in_blk = [sA[1], sB[1]]
                elif L == 2:
                    for bs1, bs2, bd in ((sA[1], sA[2], sA[0]), (sB[1], sB[2], sB[0])):
                        for c0, cl in chunks:
                            a_ = nwork.tile([P, CH], bf16, tag="resid_a")
                            nc.sync.dma_start(a_[:, :cl], bs1[:, c0:c0 + cl])
                            b_ = nwork.tile([P, CH], bf16, tag="resid_b")
                            nc.sync.dma_start(b_[:, :cl], bs2[:, c0:c0 + cl])
                            nc.vector.tensor_add(out=a_[:, :cl], in0=a_[:, :cl], in1=b_[:, :cl])
                            nc.sync.dma_start(bd[:, c0:c0 + cl], a_[:, :cl])
                    in_blk = [sA[0], sB[0]]
                else:
                    in_blk = [sA[1], sB[1]]

                # ---- node phase
                ald_sb = res.tile([P, W, 4], f32, tag=f"aldsb{L % 2}")
                if H < 4:
                    nc.vector.memset(ald_sb[:], 0.0)
                for i in range(W):
                    inT = []
                    for k in range(K):
                        it = nwork.tile([P, P], bf16, tag=f"inT{k}")
                        nc.sync.dma_start(it[:], in_blk[k][:, i * P:(i + 1) * P])
                        inT.append(it)
                    hp = psA.tile([P, P], f32, tag="hp")
                    for k in range(K):
                        nc.tensor.matmul(out=hp[:], lhsT=inT[k][:], rhs=w["gw"][:, k, :],
                                         start=(k == 0), stop=(k == K - 1))
                    hb = nwork.tile([P, P], bf16, tag="hbf")
                    nc.vector.tensor_copy(out=hb[:], in_=hp[:])
                    nc.sync.dma_start(h_bounce[i * P:(i + 1) * P, :], hb[:])
                    tm = nwork.tile([P, P], f32, tag="adtmp")
                    nc.vector.tensor_tensor(out=tm[:], in0=hb[:], in1=w["a_d"][:],
                                            op=mybir.AluOpType.mult)
                    nc.vector.reduce_sum(
                        out=ald_sb[:, i, 0:H],
                        in_=tm[:].rearrange("p (h c) -> p h c", h=H),
                        axis=mybir.AxisListType.X)
                    if L == 3:
                        xp = psB.tile([P, CH], f32, tag="x1p")
                        for k in range(K):
                            nc.tensor.matmul(out=xp[:, :P], lhsT=inT[k][:],
                                             rhs=w["fw"][:, k, :],
                                             start=(k == 0), stop=(k == K - 1))
                        x4t = nwork.tile([P, P], f32, tag="x4t")
                        nc.vector.tensor_add(out=x4t[:], in0=xp[:, :P], in1=w["fb"][:])
                        nc.vector.tensor_scalar_max(out=x4t[:], in0=x4t[:], scalar1=0.0)
                        x4b = nwork.tile([P, P], bf16, tag="x4b")
                        nc.vector.tensor_copy(out=x4b[:], in_=x4t[:])
                        nc.sync.dma_start(x4_dram[i * P:(i + 1) * P, :], x4b[:])
                ald_bf = res.tile([P, W * 4], bf16, tag=f"aldbf{L % 2}")
                nc.vector.tensor_copy(out=ald_bf[:], in_=ald_sb[:].rearrange("p w c -> p (w c)"))

                # ---- AllGather h, one call per table chunk (Shared outs)
                for ci in range(NCHUNK):
                    nc.gpsimd.collective_compute(
                        "AllGather", mybir.AluOpType.bypass,
                        replica_groups=[list(range(n_cores))],
                        ins=[h_bounce[ci * agc:(ci + 1) * agc, :]],
                        outs=[tabs[L][ci][:]],
                    )

                # ---- x1 phase (overlaps AG)
                if L < 3:
                    outA = [sA[1], sA[2], sA[1]][L]
                    for c0, cl in chunks:
                        acc = psB.tile([P, CH], f32, tag="x1p")
                        rhs = []
                        for k in range(K):
                            it = nwork.tile([P, CH], bf16, tag=f"x1in{k}")
                            nc.sync.dma_start(it[:, :cl], in_blk[k][:, c0:c0 + cl])
                            rhs.append(it)
                        for k in range(K):
                            nc.tensor.matmul(out=acc[:, :cl], lhsT=w["fw"][:, k, :],
                                             rhs=rhs[k][:, :cl],
                                             start=(k == 0), stop=(k == K - 1))
                        x1b = nwork.tile([P, CH], bf16, tag="x1b")
                        nc.scalar.activation(out=x1b[:, :cl], in_=acc[:, :cl],
                                             func=mybir.ActivationFunctionType.Relu,
                                             bias=w["fb"][:], scale=1.0)
                        nc.sync.dma_start(outA[:, c0:c0 + cl], x1b[:, :cl])

                # ---- edge phase
                for g in plan:
                    gt0, gnt = g["off"], g["ntiles"]
                    nw = len(g["windows"])
                    idxg = ework.tile([P, GNTMAX * 8], i16, tag="idxg")
                    nc.sync.dma_start(idxg[:, :gnt * 8], idx_in[:, gt0 * 8:(gt0 + gnt) * 8])
                    G = ework.tile([P, GNTMAX, P], bf16, tag="G")
                    for ci in range(NCHUNK):
                        sec = g["secs"][ci]
                        nt = sec["ntiles"]
                        if nt == 0:
                            continue
                        sl = sec["off"] - gt0
                        nc.gpsimd.dma_gather(
                            G[:, sl:sl + nt, :],
                            tabs[L][ci][:],
                            idxg[:, sl * 8:(sl + nt) * 8],
                            num_idxs=nt * P, num_idxs_reg=nt * P, elem_size=P,
                            single_packet=False)

                    als = ework.tile([P, GNTMAX * 4], f32, tag="als")
                    lg = ework.tile([P, GNTMAX * 4], f32, tag="lg")
                    for ci in range(NCHUNK):
                        sec = g["secs"][ci]
                        nt = sec["ntiles"]
                        if nt == 0:
                            continue
                        sl = sec["off"] - gt0
                        nmm = len(sec["mm"])
                        # S0T for this section
                        s0t_sb = ework.tile([P, MMMAX, P], bf16, tag="s0t")
                        nc.sync.dma_start(
                            s0t_sb[:, :nmm, :],
                            s0t_in[:].rearrange("(m e) d -> e m d", e=P)[
                                :, sec["mm0"]:sec["mm0"] + nmm, :])
                        tmp = ework.tile([P, SECMAX, P], bf16, tag="tmp")
                        nc.vector.tensor_tensor(
                            out=tmp[:, :nt, :], in0=G[:, sl:sl + nt, :],
                            in1=w["a_s"][:].rearrange("p q -> p () q").to_broadcast([P, nt, P]),
                            op=mybir.AluOpType.mult)
                        nc.vector.reduce_sum(
                            out=als[:, sl * H:(sl + nt) * H],
                            in_=tmp[:, :nt, :].rearrange("p t (h c) -> p (t h) c", h=H),
                            axis=mybir.AxisListType.X)
                        # ald per edge: accumulate over mm entries per tile
                        aldp = psA.tile([P, SECMAX, 4], f32, tag="ald", bufs=1)
                        tile_first = {}
                        tile_last = {}
                        for mi, (t, wl) in enumerate(sec["mm"]):
                            tile_first.setdefault(t, mi)
                            tile_last[t] = mi
                        for mi, (t, wl) in enumerate(sec["mm"]):
                            wi = g["windows"][wl]
                            nc.tensor.matmul(
                                out=aldp[:, t, 0:H],
                                lhsT=s0t_sb[:, mi, :],
                                rhs=ald_bf[:, wi * 4:wi * 4 + H],
                                start=(tile_first[t] == mi),
                                stop=(tile_last[t] == mi))
                        nc.vector.tensor_add(
                            out=lg[:, sl * H:(sl + nt) * H],
                            in0=als[:, sl * H:(sl + nt) * H],
                            in1=aldp[:, :nt, 0:H].rearrange("p t h -> p (t h)"))

                    lr = ework.tile([P, GNTMAX * 4], f32, tag="lr")
                    nc.vector.scalar_tensor_tensor(
                        out=lr[:, :gnt * H], in0=lg[:, :gnt * H], scalar=0.2,
                        in1=lg[:, :gnt * H],
                        op0=mybir.AluOpType.mult, op1=mybir.AluOpType.max)
                    pe_t = ework.tile([P, GNTMAX * 4], f32, tag="pe")
                    nc.scalar.activation(out=pe_t[:, :gnt * H], in_=lr[:, :gnt * H],
                                         func=mybir.ActivationFunctionType.Exp)

                    x2acc = ework.tile([P, GB, P + 4], f32, tag="x2acc")
                    wdone = {}
                    for ci in range(NCHUNK):
                        sec = g["secs"][ci]
                        nt = sec["ntiles"]
                        if nt == 0:
                            continue
                        sl = sec["off"] - gt0
                        nmm = len(sec["mm"])
                        s0_sb = ework.tile([P, MMMAX, P], bf16, tag="s0")
                        nc.sync.dma_start(
                            s0_sb[:, :nmm, :],
                            s0_in[:].rearrange("(m e) d -> e m d", e=P)[
                                :, sec["mm0"]:sec["mm0"] + nmm, :])
                        GW = ework.tile([P, SECMAX, P + 4], bf16, tag="GW")
                        nc.vector.tensor_tensor(
                            out=GW[:, :nt, 0:P].rearrange("p t (h c) -> p t h c", h=H),
                            in0=G[:, sl:sl + nt, :].rearrange("p t (h c) -> p t h c", h=H),
                            in1=pe_t[:, sl * H:(sl + nt) * H]
                                .rearrange("p (t h) -> p t h ()", h=H)
                                .to_broadcast([P, nt, H, C]),
                            op=mybir.AluOpType.mult)
                        nc.vector.tensor_copy(
                            out=GW[:, :nt, P:P + H],
                            in_=pe_t[:, sl * H:(sl + nt) * H].rearrange("p (t h) -> p t h", h=H))
                        # aggregate per window: entries sorted by (wl, tile)
                        mm = sec["mm"]
                        mi = 0
                        while mi < len(mm):
                            wl = mm[mi][1]
                            mj = mi
                            while mj < len(mm) and mm[mj][1] == wl:
                                mj += 1
                            aggp = psA.tile([P, P + 4], f32, tag="agg")
                            for k in range(mi, mj):
                                t = mm[k][0]
                                nc.tensor.matmul(
                                    out=aggp[:, :P + H],
                                    lhsT=s0_sb[:, k, :],
                                    rhs=GW[:, t, 0:P + H],
                                    start=(k == mi), stop=(k == mj - 1))
                            wi = g["windows"][wl]
                            if wi not in wdone:
                                wdone[wi] = True
                                nc.vector.tensor_copy(
                                    out=x2acc[:, wl, 0:P + H], in_=aggp[:, :P + H])
                            else:
                                nc.vector.tensor_add(
                                    out=x2acc[:, wl, 0:P + H],
                                    in0=x2acc[:, wl, 0:P + H], in1=aggp[:, :P + H])
                            mi = mj

                    for wl, wi in enumerate(g["windows"]):
                        sinv = ework.tile([P, 4], f32, tag="sinv")
                        nc.vector.tensor_scalar_add(
                            out=sinv[:, :H], in0=x2acc[:, wl, P:P + H], scalar1=1e-16)
                        nc.vector.reciprocal(out=sinv[:, :H], in_=sinv[:, :H])
                        x2 = ework.tile([P, P], f32, tag="x2")
                        nc.vector.tensor_tensor(
                            out=x2[:].rearrange("p (h c) -> p h c", h=H),
                            in0=x2acc[:, wl, 0:P].rearrange("p (h c) -> p h c", h=H),
                            in1=sinv[:, :H].rearrange("p h -> p h ()").to_broadcast([P, H, C]),
                            op=mybir.AluOpType.mult)
                        nc.vector.tensor_add(out=x2[:], in0=x2[:], in1=w["gb"][:])
                        nc.vector.tensor_scalar_max(out=x2[:], in0=x2[:], scalar1=0.0)
                        if L < 3:
                            x2b = ework.tile([P, P], bf16, tag="x2b")
                            nc.vector.tensor_copy(out=x2b[:], in_=x2[:])
                            tp = psB.tile([P, P], bf16, tag="tp", bufs=1)
                            nc.tensor.transpose(out=tp[:], in_=x2b[:], identity=ident[:])
                            x2T = ework.tile([P, P], bf16, tag="x2T")
                            nc.vector.tensor_copy(out=x2T[:], in_=tp[:])
                            outB = [sB[1], sB[2], sB[1]][L]
                            nc.sync.dma_start(outB[:, wi * P:(wi + 1) * P], x2T[:])
                        else:
                            x4t = ework.tile([P, P], bf16, tag="x4in")
                            nc.sync.dma_start(x4t[:], x4_dram[wi * P:(wi + 1) * P, :])
                            yo = ework.tile([P, P], f32, tag="yo")
                            nc.vector.tensor_add(out=yo[:], in0=x2[:], in1=x4t[:])
                            nc.sync.dma_start(out_t[wi * P:(wi + 1) * P, :], yo[:])

    nc.compile()
    return nc


# --------------------------------------------------------------------------
# runner
# --------------------------------------------------------------------------
def make_in_maps(inputs, st):
    x = np.asarray(inputs["x"], np.float32)
    shard, SH = st["shard"], st["SH"]
    layers = prep_weights(inputs)
    iota8 = np.broadcast_to(np.arange(P, dtype=np.int8), (P, P)).copy()
    iotac8 = np.arange(P, dtype=np.int8).reshape(P, 1).copy()

    common = {"iota8": iota8, "iotac8": iotac8}
    for L, lwd in enumerate(layers):
        common[f"gw{L}"] = lwd["gw"]
        common[f"fw{L}"] = lwd["fw"]
        common[f"as{L}"] = lwd["a_s"]
        common[f"ad{L}"] = lwd["a_d"]
        common[f"gb{L}"] = lwd["gb"]
        common[f"fb{L}"] = lwd["fb"]

    in_maps = []
    for c in range(N_CORES):
        xs = np.zeros((SH, P), np.float32)
        xs[:shard] = x[c * shard:(c + 1) * shard]
        m = dict(common)
        m["x"] = xs
        m["idx"] = st["idx"][c]
        m["S0"] = st["S0"][c]
        m["S0T"] = st["S0T"][c]
        in_maps.append(m)
    return in_maps


_CACHE = {}


def run(inputs, trace=False):
    from concourse.bass_utils import run_bass_kernel_spmd

    st = prep_structure(np.asarray(inputs["edge_index"]), N_NODES, N_CORES)
    key = (st["SH"], st["TT"], st["NMM"])
    if key not in _CACHE:
        _CACHE[key] = build_program(st, N_CORES)
    nc = _CACHE[key]
    in_maps = make_in_maps(inputs, st)
    res = run_bass_kernel_spmd(nc, in_maps, core_ids=list(range(N_CORES)),
                               trace=trace)
    outs = [np.asarray(res.results[c]["out"])[:st["shard"]] for c in range(N_CORES)]
    return np.concatenate(outs, axis=0).astype(np.float32), res


def kernel(**inputs):
    out, _ = run(inputs, trace=False)
    return out



# revision 13
# speedup vs baseline: 1.6217x; 1.6217x over previous
"""Trainium2 Bass kernel for DirectedNetworkFeatureExtractor (GAT+FC GNN), v2.

Structure (per layer, nodes sharded 8-way, transposed-state free):
  node phase: h = x @ Wg (PSUM), als/ald reductions, h -> h_bounce DRAM
  AllGather h table (bf16, 8 x 1568-row chunks, Shared out)
  edge phase per (window-group, chunk) section:
    dma_gather h[src] (256B rows, int16 idx, 4 table chunks)
    als per edge (G * a_s reduce), ald per edge via host-built one-hot S0T
    matmuls, p = exp(leaky_relu(als+ald)), segment-sum via host-built S0
    one-hot matmuls accumulating (sum p*h, sum p) per dst window in PSUM
  x2 = agg/sum_p + bias, relu; x1 = fc branch; concat -> next layer state.

v2 vs v1: S0/S0T one-hots precomputed on host (DRAM inputs, DMA-streamed),
window-impure gather tiles (fewer padded slots), AGC=1568, Shared AG out.
"""
import math
import sys

sys.path.insert(0, "/opt/trn_rl_repo")

import numpy as np
import ml_dtypes

import concourse.bass as bass
import concourse.bacc as bacc
import concourse.tile as tile
from concourse import mybir

BF = ml_dtypes.bfloat16
P = 128

N_NODES = 100_000
N_CORES = 8
HEADS = 4
NCHUNK = 4
GB = 4          # windows per gather group
AGC = 3136      # AllGather chunk rows (= CHSZ/8 so one AG = one table chunk)


# --------------------------------------------------------------------------
# host-side graph preprocessing (untimed)
# --------------------------------------------------------------------------
def prep_structure(edge_index, n_nodes, n_cores):
    src = np.asarray(edge_index[0]).astype(np.int64)
    dst = np.asarray(edge_index[1]).astype(np.int64)
    shard = n_nodes // n_cores
    W = math.ceil(shard / P)
    SH = W * P
    TAB = n_cores * SH
    CHSZ = TAB // NCHUNK
    NE = len(src)

    core = dst // shard
    dloc = dst - core * shard
    win = dloc // P
    # AG-chunk-major table layout
    agc = AGC
    assert SH % agc == 0 and CHSZ % (agc * n_cores) == 0
    loc = src % shard
    rnk = src // shard
    a0 = loc // agc
    grow = (a0 * n_cores + rnk) * agc + (loc % agc)   # table row
    chk = grow // CHSZ
    rel = (grow - chk * CHSZ).astype(np.int64)

    NG = math.ceil(W / GB)
    g_of_w = np.arange(W) // GB

    # section (g, c) sizes pooled across windows, max over cores
    cnt = np.zeros((n_cores, NG, NCHUNK), np.int64)
    np.add.at(cnt, (core, g_of_w[win], chk), 1)
    sec_tiles = (cnt.max(axis=0) + P - 1) // P        # [NG, NCHUNK]

    # global tile base per section; plan in (g, then c) order
    plan = []
    t_run = 0
    for g in range(NG):
        ws = list(range(g * GB, min((g + 1) * GB, W)))
        g_off = t_run
        secs = []
        for c in range(NCHUNK):
            nt = int(sec_tiles[g, c])
            secs.append(dict(off=t_run, ntiles=nt, mm=None))
            t_run += nt
        plan.append(dict(windows=ws, off=g_off,
                         ntiles=t_run - g_off, secs=secs))
    TT = t_run

    idx16 = np.zeros((n_cores, TT * P), np.int16)

    # per-core slot assignment: edges sorted by (group, chunk, window, dloc)
    order = np.lexsort((np.arange(NE), dloc, chk, g_of_w[win], core))
    s_core = core[order]
    s_chk = chk[order]
    s_g = g_of_w[win[order]]
    s_win = win[order]
    s_rel = rel[order]
    s_dloc = dloc[order]

    # position within each (core, g, c) run
    gid = (s_core * NG + s_g) * NCHUNK + s_chk
    first = np.r_[True, gid[1:] != gid[:-1]]
    starts = np.flatnonzero(first)
    run_id = np.cumsum(first) - 1
    pos = np.arange(NE) - starts[run_id]

    sec_base = np.zeros((NG, NCHUNK), np.int64)
    for g in range(NG):
        for c in range(NCHUNK):
            sec_base[g, c] = plan[g]["secs"][c]["off"]
    tile_i = sec_base[s_g, s_chk] + pos // P
    part_i = pos % P
    idx16[s_core, tile_i * P + part_i] = s_rel.astype(np.int16)

    # slot -> (window, local slot) per core; -1 window means pad
    slot_win = np.full((n_cores, TT * P), -1, np.int8)   # window - g*GB
    slot_d = np.zeros((n_cores, TT * P), np.int8)        # dloc % 128
    slot_win[s_core, tile_i * P + part_i] = (s_win - s_g * GB).astype(np.int8)
    slot_d[s_core, tile_i * P + part_i] = (s_dloc % P).astype(np.int8)

    # ---- MM entries per section: (tile, local window) pairs with edges.
    # Shared across cores: entry exists if ANY core has edges there; S0/S0T
    # content is per-core.  Entry order: per section, by (window, tile).
    entries = []           # list of (g, c, tile_global, wl)
    for g in range(NG):
        nw = len(plan[g]["windows"])
        for ci in range(NCHUNK):
            sec = plan[g]["secs"][ci]
            nt = sec["ntiles"]
            t0 = sec["off"]
            if nt == 0:
                sec["mm"] = []
                sec["mm0"] = len(entries)
                continue
            # which (tile, wl) pairs are nonempty on any core
            sw = slot_win[:, t0 * P:(t0 + nt) * P].reshape(n_cores, nt, P)
            mm = []
            for wl in range(nw):
                tiles = np.flatnonzero((sw == wl).any(axis=(0, 2)))
                for t in tiles:
                    mm.append((int(t), wl))
            # order by tile then window for PSUM run grouping by window:
            # group by window (runs), tiles ascending within run
            mm.sort(key=lambda x: (x[1], x[0]))
            sec["mm"] = mm
            sec["mm0"] = len(entries)
            for (t, wl) in mm:
                entries.append((g, ci, t0 + t, wl))
    NMM = len(entries)

    # host one-hots per core: S0 [NMM, P(e), P(d)], S0T transposed
    S0 = np.zeros((n_cores, NMM, P, P), BF)
    e_idx = np.arange(P)
    for mi, (g, ci, tg, wl) in enumerate(entries):
        for cr in range(n_cores):
            swv = slot_win[cr, tg * P:(tg + 1) * P]
            sdv = slot_d[cr, tg * P:(tg + 1) * P]
            m = swv == wl
            if m.any():
                S0[cr, mi, e_idx[m], sdv[m].astype(np.int64)] = 1.0
    # pack idx for dma_gather: element i at [r, i//16] for r%16 == i%16
    j = np.arange(TT * 8)
    r16 = np.arange(16)
    packed = idx16[:, (j[None, :] * 16 + r16[:, None]).reshape(16, -1)]
    idx_packed = np.tile(packed, (1, 8, 1))       # [cores, 128, TT*8]

    # partition-major one-hot streams: S0pm[c][e, mi*P+d], S0Tpm[c][d, mi*P+e]
    S0pm = np.ascontiguousarray(
        np.transpose(S0, (0, 2, 1, 3)).reshape(n_cores, P, NMM * P))
    S0Tpm = np.ascontiguousarray(
        np.transpose(S0, (0, 3, 1, 2)).reshape(n_cores, P, NMM * P))

    return dict(
        shard=shard, W=W, SH=SH, TT=TT, TAB=TAB, CHSZ=CHSZ, NG=NG,
        plan=plan, NMM=NMM, agc=agc,
        idx=np.ascontiguousarray(idx_packed),
        S0=S0pm,
        S0T=S0Tpm,
    )


def prep_weights(inputs):
    def blocks(w):
        k = w.shape[0]
        return np.ascontiguousarray(w.reshape(k // P, P, w.shape[1]).astype(BF))

    def rep_row(v):
        return np.broadcast_to(np.asarray(v, np.float32), (P, P)).copy()

    g = lambda n: np.asarray(inputs[n], np.float32)
    layers = [dict(
        gw=blocks(g("g1_W")), fw=blocks(g("fc1_W")),
        a_s=rep_row(g("g1_as").reshape(-1)).astype(BF),
        a_d=rep_row(g("g1_ad").reshape(-1)).astype(BF),
        gb=rep_row(g("g1_b")), fb=g("fc1_b").reshape(P, 1).astype(np.float32),
    )]
    for i in range(2):
        layers.append(dict(
            gw=blocks(g("mg_W")[i]), fw=blocks(g("mfc_W")[i]),
            a_s=rep_row(g("mg_as")[i].reshape(-1)).astype(BF),
            a_d=rep_row(g("mg_ad")[i].reshape(-1)).astype(BF),
            gb=rep_row(g("mg_b")[i]), fb=g("mfc_b")[i].reshape(P, 1).astype(np.float32),
        ))
    layers.append(dict(
        gw=blocks(g("fg_W")), fw=blocks(g("ffc_W")),
        a_s=rep_row(g("fg_as").reshape(-1)).astype(BF),
        a_d=rep_row(g("fg_ad").reshape(-1)).astype(BF),
        gb=rep_row(g("fg_b")), fb=rep_row(g("ffc_b")),
    ))
    return layers


# --------------------------------------------------------------------------
# device program
# --------------------------------------------------------------------------
def build_program(st, n_cores):
    SH, W, TT, TAB, CHSZ = st["SH"], st["W"], st["TT"], st["TAB"], st["CHSZ"]
    agc = st["agc"]
    plan = st["plan"]
    NMM = st["NMM"]
    dt = mybir.dt
    f32, bf16, i16, i8 = dt.float32, dt.bfloat16, dt.int16, dt.int8
    HL = [HEADS, HEADS, HEADS, 1]
    GNTMAX = max(g["ntiles"] for g in plan)
    SECMAX = max(s["ntiles"] for g in plan for s in g["secs"])
    MMMAX = max(len(s["mm"]) for g in plan for s in g["secs"])

    nc = bacc.Bacc(None, num_swdge_queues=4)

    def inp(name, shape, d):
        return nc.declare_dram_parameter(name, list(shape), d, isOutput=False)

    x_in = inp("x", (SH, P), f32)
    idx_in = inp("idx", (P, TT * 8), i16)
    s0_in = inp("S0", (P, NMM * P), bf16)
    s0t_in = inp("S0T", (P, NMM * P), bf16)
    iota_in = inp("iota8", (P, P), i8)
    iotac_in = inp("iotac8", (P, 1), i8)
    lw = []
    for L in range(4):
        K = 1 if L == 0 else 2
        lw.append(dict(
            gw=inp(f"gw{L}", (K, P, P), bf16),
            fw=inp(f"fw{L}", (K, P, P), bf16),
            a_s=inp(f"as{L}", (P, P), bf16),
            a_d=inp(f"ad{L}", (P, P), bf16),
            gb=inp(f"gb{L}", (P, P), f32),
            fb=inp(f"fb{L}", (P, 1) if L < 3 else (P, P), f32),
        ))
    out_t = nc.declare_dram_parameter("out", [SH, P], f32, isOutput=True)

    CH = 512
    chunks = [(c, min(CH, SH - c)) for c in range(0, SH, CH)]

    with tile.TileContext(nc) as tc:
        with (
            tc.tile_pool(name="res", bufs=1) as res,
            tc.tile_pool(name="wts", bufs=1) as wts,
            tc.tile_pool(name="nwork", bufs=3) as nwork,
            tc.tile_pool(name="ework", bufs=2) as ework,
            tc.tile_pool(name="psA", bufs=2, space="PSUM") as psA,
            tc.tile_pool(name="psB", bufs=2, space="PSUM") as psB,
            tc.tile_pool(name="dram", bufs=1, space="DRAM") as dram,
        ):
            # ---------------- residents
            iota8 = res.tile([P, P], i8)
            nc.sync.dma_start(iota8[:], iota_in[:])
            iotac8 = res.tile([P, 1], i8)
            nc.sync.dma_start(iotac8[:], iotac_in[:])
            ident = res.tile([P, P], bf16)
            nc.vector.tensor_tensor(
                out=ident[:], in0=iotac8[:].to_broadcast([P, P]), in1=iota8[:],
                op=mybir.AluOpType.is_equal)

            wt = []
            for L in range(4):
                K = 1 if L == 0 else 2
                d = {}
                for nm in ("gw", "fw"):
                    t_ = wts.tile([P, K, P], bf16, tag=f"{nm}{L}")
                    nc.sync.dma_start(t_[:], lw[L][nm][:].rearrange("k p q -> p k q"))
                    d[nm] = t_
                for nm in ("a_s", "a_d"):
                    t_ = wts.tile([P, P], bf16, tag=f"{nm}{L}")
                    nc.sync.dma_start(t_[:], lw[L][nm][:])
                    d[nm] = t_
                t_ = wts.tile([P, P], f32, tag=f"gb{L}")
                nc.sync.dma_start(t_[:], lw[L]["gb"][:])
                d["gb"] = t_
                t_ = wts.tile([P, 1] if L < 3 else [P, P], f32, tag=f"fb{L}")
                nc.sync.dma_start(t_[:], lw[L]["fb"][:])
                d["fb"] = t_
                wt.append(d)

            # ---------------- DRAM scratch
            sA = [dram.tile([P, SH], bf16, tag=f"sA{i}", name=f"sA{i}") for i in range(3)]
            sB = [dram.tile([P, SH], bf16, tag=f"sB{i}", name=f"sB{i}") for i in range(3)]
            h_bounce = dram.tile([SH, P], bf16, tag="hb")
            tabs = [[dram.tile([CHSZ, P], bf16, tag=f"tab{L}_{ci}",
                               name=f"tab{L}_{ci}", addr_space="Shared")
                     for ci in range(NCHUNK)] for L in range(4)]
            x4_dram = dram.tile([SH, P], bf16, tag="x4")

            # ---------------- x -> transposed state
            for i in range(W):
                xt = nwork.tile([P, P], f32, tag="xin")
                nc.sync.dma_start(xt[:], x_in[i * P:(i + 1) * P, :])
                xb = nwork.tile([P, P], bf16, tag="xbf")
                nc.vector.tensor_copy(out=xb[:], in_=xt[:])
                tp = psB.tile([P, P], bf16, tag="tp", bufs=1)
                nc.tensor.transpose(out=tp[:], in_=xb[:], identity=ident[:])
                xTb = nwork.tile([P, P], bf16, tag="xT")
                nc.vector.tensor_copy(out=xTb[:], in_=tp[:])
                nc.sync.dma_start(sA[0][:, i * P:(i + 1) * P], xTb[:])

            # ---------------- layers
            for L in range(4):
                K = 1 if L == 0 else 2
                H = HL[L]
                C = P // H
                w = wt[L]
                if L == 0:
                    in_blk = [sA[0]]
                elif L == 1:
                    in_blk = [sA[1], sB[1]]
                elif L == 2:
                    for bs1, bs2, bd in ((sA[1], sA[2], sA[0]), (sB[1], sB[2], sB[0])):
                        for c0, cl in chunks:
                            a_ = nwork.tile([P, CH], bf16, tag="resid_a")
                            nc.sync.dma_start(a_[:, :cl], bs1[:, c0:c0 + cl])
                            b_ = nwork.tile([P, CH], bf16, tag="resid_b")
                            nc.sync.dma_start(b_[:, :cl], bs2[:, c0:c0 + cl])
                            nc.vector.tensor_add(out=a_[:, :cl], in0=a_[:, :cl], in1=b_[:, :cl])
                            nc.sync.dma_start(bd[:, c0:c0 + cl], a_[:, :cl])
                    in_blk = [sA[0], sB[0]]
                else:
                    in_blk = [sA[1], sB[1]]

                # ---- node phase
                ald_sb = res.tile([P, W, 4], f32, tag=f"aldsb{L % 2}")
                if H < 4:
                    nc.vector.memset(ald_sb[:], 0.0)
                for i in range(W):
                    inT = []
                    for k in range(K):
                        it = nwork.tile([P, P], bf16, tag=f"inT{k}")
                        nc.sync.dma_start(it[:], in_blk[k][:, i * P:(i + 1) * P])
                        inT.append(it)
                    hp = psA.tile([P, P], f32, tag="hp")
                    for k in range(K):
                        nc.tensor.matmul(out=hp[:], lhsT=inT[k][:], rhs=w["gw"][:, k, :],
                                         start=(k == 0), stop=(k == K - 1))
                    hb = nwork.tile([P, P], bf16, tag="hbf")
                    nc.vector.tensor_copy(out=hb[:], in_=hp[:])
                    nc.sync.dma_start(h_bounce[i * P:(i + 1) * P, :], hb[:])
                    tm = nwork.tile([P, P], f32, tag="adtmp")
                    nc.vector.tensor_tensor(out=tm[:], in0=hb[:], in1=w["a_d"][:],
                                            op=mybir.AluOpType.mult)
                    nc.vector.reduce_sum(
                        out=ald_sb[:, i, 0:H],
                        in_=tm[:].rearrange("p (h c) -> p h c", h=H),
                        axis=mybir.AxisListType.X)
                    if L == 3:
                        xp = psB.tile([P, CH], f32, tag="x1p")
                        for k in range(K):
                            nc.tensor.matmul(out=xp[:, :P], lhsT=inT[k][:],
                                             rhs=w["fw"][:, k, :],
                                             start=(k == 0), stop=(k == K - 1))
                        x4t = nwork.tile([P, P], f32, tag="x4t")
                        nc.vector.tensor_add(out=x4t[:], in0=xp[:, :P], in1=w["fb"][:])
                        nc.vector.tensor_scalar_max(out=x4t[:], in0=x4t[:], scalar1=0.0)
                        x4b = nwork.tile([P, P], bf16, tag="x4b")
                        nc.vector.tensor_copy(out=x4b[:], in_=x4t[:])
                        nc.sync.dma_start(x4_dram[i * P:(i + 1) * P, :], x4b[:])
                ald_bf = res.tile([P, W * 4], bf16, tag=f"aldbf{L % 2}")
                nc.vector.tensor_copy(out=ald_bf[:], in_=ald_sb[:].rearrange("p w c -> p (w c)"))

                # ---- AllGather h, one call per table chunk (Shared outs)
                for ci in range(NCHUNK):
                    nc.gpsimd.collective_compute(
                        "AllGather", mybir.AluOpType.bypass,
                        replica_groups=[list(range(n_cores))],
                        ins=[h_bounce[ci * agc:(ci + 1) * agc, :]],
                        outs=[tabs[L][ci][:]],
                    )

                # ---- x1 phase (overlaps AG)
                if L < 3:
                    outA = [sA[1], sA[2], sA[1]][L]
                    for c0, cl in chunks:
                        acc = psB.tile([P, CH], f32, tag="x1p")
                        rhs = []
                        for k in range(K):
                            it = nwork.tile([P, CH], bf16, tag=f"x1in{k}")
                            nc.sync.dma_start(it[:, :cl], in_blk[k][:, c0:c0 + cl])
                            rhs.append(it)
                        for k in range(K):
                            nc.tensor.matmul(out=acc[:, :cl], lhsT=w["fw"][:, k, :],
                                             rhs=rhs[k][:, :cl],
                                             start=(k == 0), stop=(k == K - 1))
                        x1b = nwork.tile([P, CH], bf16, tag="x1b")
                        nc.scalar.activation(out=x1b[:, :cl], in_=acc[:, :cl],
                                             func=mybir.ActivationFunctionType.Relu,
                                             bias=w["fb"][:], scale=1.0)
                        nc.sync.dma_start(outA[:, c0:c0 + cl], x1b[:, :cl])

                # ---- edge phase
                for g in plan:
                    gt0, gnt = g["off"], g["ntiles"]
                    nw = len(g["windows"])
                    idxg = ework.tile([P, GNTMAX * 8], i16, tag="idxg")
                    nc.sync.dma_start(idxg[:, :gnt * 8], idx_in[:, gt0 * 8:(gt0 + gnt) * 8])
                    G = ework.tile([P, GNTMAX, P], bf16, tag="G")
                    for ci in range(NCHUNK):
                        sec = g["secs"][ci]
                        nt = sec["ntiles"]
                        if nt == 0:
                            continue
                        sl = sec["off"] - gt0
                        nc.gpsimd.dma_gather(
                            G[:, sl:sl + nt, :],
                            tabs[L][ci][:],
                            idxg[:, sl * 8:(sl + nt) * 8],
                            num_idxs=nt * P, num_idxs_reg=nt * P, elem_size=P,
                            single_packet=False, queue_num=ci)

                    als = ework.tile([P, GNTMAX * 4], bf16, tag="als")
                    lg = ework.tile([P, GNTMAX * 4], f32, tag="lg")
                    for ci in range(NCHUNK):
                        sec = g["secs"][ci]
                        nt = sec["ntiles"]
                        if nt == 0:
                            continue
                        sl = sec["off"] - gt0
                        nmm = len(sec["mm"])
                        # S0T for this section (partition-major: [d, m, e])
                        s0t_sb = ework.tile([P, MMMAX, P], bf16, tag="s0t")
                        nc.sync.dma_start(
                            s0t_sb[:, :nmm, :],
                            s0t_in[:, sec["mm0"] * P:(sec["mm0"] + nmm) * P]
                            .rearrange("d (m e) -> d m e", m=nmm))
                        tmp = ework.tile([P, SECMAX, P], bf16, tag="tmp")
                        nc.vector.tensor_tensor(
                            out=tmp[:, :nt, :], in0=G[:, sl:sl + nt, :],
                            in1=w["a_s"][:].rearrange("p q -> p () q").to_broadcast([P, nt, P]),
                            op=mybir.AluOpType.mult)
                        with nc.allow_low_precision(reason="als logits tolerate bf16"):
                            nc.vector.reduce_sum(
                                out=als[:, sl * H:(sl + nt) * H],
                                in_=tmp[:, :nt, :].rearrange("p t (h c) -> p (t h) c", h=H),
                                axis=mybir.AxisListType.X)
                        # ald per edge: accumulate over mm entries per tile
                        aldp = psA.tile([P, SECMAX, 4], f32, tag="ald", bufs=1)
                        tile_first = {}
                        tile_last = {}
                        for mi, (t, wl) in enumerate(sec["mm"]):
                            tile_first.setdefault(t, mi)
                            tile_last[t] = mi
                        for mi, (t, wl) in enumerate(sec["mm"]):
                            wi = g["windows"][wl]
                            nc.tensor.matmul(
                                out=aldp[:, t, 0:H],
                                lhsT=s0t_sb[:, mi, :],
                                rhs=ald_bf[:, wi * 4:wi * 4 + H],
                                start=(tile_first[t] == mi),
                                stop=(tile_last[t] == mi))
                        nc.vector.tensor_add(
                            out=lg[:, sl * H:(sl + nt) * H],
                            in0=als[:, sl * H:(sl + nt) * H],
                            in1=aldp[:, :nt, 0:H].rearrange("p t h -> p (t h)"))

                    lr = ework.tile([P, GNTMAX * 4], f32, tag="lr")
                    nc.vector.scalar_tensor_tensor(
                        out=lr[:, :gnt * H], in0=lg[:, :gnt * H], scalar=0.2,
                        in1=lg[:, :gnt * H],
                        op0=mybir.AluOpType.mult, op1=mybir.AluOpType.max)
                    pe_t = ework.tile([P, GNTMAX * 4], bf16, tag="pe")
                    nc.scalar.activation(out=pe_t[:, :gnt * H], in_=lr[:, :gnt * H],
                                         func=mybir.ActivationFunctionType.Exp)

                    x2acc = ework.tile([P, GB, P + 4], f32, tag="x2acc")
                    wdone = {}
                    for ci in range(NCHUNK):
                        sec = g["secs"][ci]
                        nt = sec["ntiles"]
                        if nt == 0:
                            continue
                        sl = sec["off"] - gt0
                        nmm = len(sec["mm"])
                        s0_sb = ework.tile([P, MMMAX, P], bf16, tag="s0")
                        nc.sync.dma_start(
                            s0_sb[:, :nmm, :],
                            s0_in[:, sec["mm0"] * P:(sec["mm0"] + nmm) * P]
                            .rearrange("e (m d) -> e m d", m=nmm))
                        GW = ework.tile([P, SECMAX, P + 4], bf16, tag="GW")
                        nc.vector.tensor_tensor(
                            out=GW[:, :nt, 0:P].rearrange("p t (h c) -> p t h c", h=H),
                            in0=G[:, sl:sl + nt, :].rearrange("p t (h c) -> p t h c", h=H),
                            in1=pe_t[:, sl * H:(sl + nt) * H]
                                .rearrange("p (t h) -> p t h ()", h=H)
                                .to_broadcast([P, nt, H, C]),
                            op=mybir.AluOpType.mult)
                        nc.vector.tensor_copy(
                            out=GW[:, :nt, P:P + H],
                            in_=pe_t[:, sl * H:(sl + nt) * H].rearrange("p (t h) -> p t h", h=H))
                        # aggregate per window: entries sorted by (wl, tile)
                        mm = sec["mm"]
                        mi = 0
                        while mi < len(mm):
                            wl = mm[mi][1]
                            mj = mi
                            while mj < len(mm) and mm[mj][1] == wl:
                                mj += 1
                            aggp = psA.tile([P, P + 4], f32, tag="agg")
                            for k in range(mi, mj):
                                t = mm[k][0]
                                nc.tensor.matmul(
                                    out=aggp[:, :P + H],
                                    lhsT=s0_sb[:, k, :],
                                    rhs=GW[:, t, 0:P + H],
                                    start=(k == mi), stop=(k == mj - 1))
                            wi = g["windows"][wl]
                            if wi not in wdone:
                                wdone[wi] = True
                                nc.vector.tensor_copy(
                                    out=x2acc[:, wl, 0:P + H], in_=aggp[:, :P + H])
                            else:
                                nc.vector.tensor_add(
                                    out=x2acc[:, wl, 0:P + H],
                                    in0=x2acc[:, wl, 0:P + H], in1=aggp[:, :P + H])
                            mi = mj

                    # batched window finish for the whole group
                    nw = len(g["windows"])
                    wi0 = g["windows"][0]
                    sinv = ework.tile([P, GB, 4], f32, tag="sinv")
                    nc.vector.tensor_scalar_add(
                        out=sinv[:, :nw, 0:H], in0=x2acc[:, :nw, P:P + H],
                        scalar1=1e-16)
                    nc.vector.reciprocal(out=sinv[:, :nw, 0:H],
                                         in_=sinv[:, :nw, 0:H])
                    x2g = ework.tile([P, GB, P], f32, tag="x2g")
                    nc.vector.tensor_tensor(
                        out=x2g[:, :nw, :].rearrange("p w (h c) -> p w h c", h=H),
                        in0=x2acc[:, :nw, 0:P].rearrange("p w (h c) -> p w h c", h=H),
                        in1=sinv[:, :nw, 0:H].rearrange("p w h -> p w h ()")
                            .to_broadcast([P, nw, H, C]),
                        op=mybir.AluOpType.mult)
                    nc.vector.tensor_add(
                        out=x2g[:, :nw, :], in0=x2g[:, :nw, :],
                        in1=w["gb"][:].rearrange("p q -> p () q")
                            .to_broadcast([P, nw, P]))
                    nc.vector.tensor_scalar_max(
                        out=x2g[:, :nw, :], in0=x2g[:, :nw, :], scalar1=0.0)
                    if L < 3:
                        x2b = ework.tile([P, GB, P], bf16, tag="x2b")
                        nc.vector.tensor_copy(out=x2b[:, :nw, :], in_=x2g[:, :nw, :])
                        outB = [sB[1], sB[2], sB[1]][L]
                        for wl, wi in enumerate(g["windows"]):
                            tp = psB.tile([P, P], bf16, tag="tp", bufs=1)
                            nc.tensor.transpose(out=tp[:], in_=x2b[:, wl, :],
                                                identity=ident[:])
                            x2T = ework.tile([P, P], bf16, tag="x2T")
                            nc.vector.tensor_copy(out=x2T[:], in_=tp[:])
                            nc.sync.dma_start(outB[:, wi * P:(wi + 1) * P], x2T[:])
                    else:
                        x4t = ework.tile([P, GB, P], bf16, tag="x4in")
                        nc.sync.dma_start(
                            x4t[:, :nw, :],
                            x4_dram[wi0 * P:(wi0 + nw) * P, :]
                            .rearrange("(w p) q -> p w q", p=P))
                        yo = ework.tile([P, GB, P], f32, tag="yo")
                        nc.vector.tensor_add(out=yo[:, :nw, :],
                                             in0=x2g[:, :nw, :], in1=x4t[:, :nw, :])
                        nc.sync.dma_start(
                            out_t[wi0 * P:(wi0 + nw) * P, :]
                            .rearrange("(w p) q -> p w q", p=P),
                            yo[:, :nw, :])

    nc.compile()
    return nc


# --------------------------------------------------------------------------
# runner
# --------------------------------------------------------------------------
def make_in_maps(inputs, st):
    x = np.asarray(inputs["x"], np.float32)
    shard, SH = st["shard"], st["SH"]
    layers = prep_weights(inputs)
    iota8 = np.broadcast_to(np.arange(P, dtype=np.int8), (P, P)).copy()
    iotac8 = np.arange(P, dtype=np.int8).reshape(P, 1).copy()

    common = {"iota8": iota8, "iotac8": iotac8}
    for L, lwd in enumerate(layers):
        common[f"gw{L}"] = lwd["gw"]
        common[f"fw{L}"] = lwd["fw"]
        common[f"as{L}"] = lwd["a_s"]
        common[f"ad{L}"] = lwd["a_d"]
        common[f"gb{L}"] = lwd["gb"]
        common[f"fb{L}"] = lwd["fb"]

    in_maps = []
    for c in range(N_CORES):
        xs = np.zeros((SH, P), np.float32)
        xs[:shard] = x[c * shard:(c + 1) * shard]
        m = dict(common)
        m["x"] = xs
        m["idx"] = st["idx"][c]
        m["S0"] = st["S0"][c]
        m["S0T"] = st["S0T"][c]
        in_maps.append(m)
    return in_maps


_CACHE = {}


def run(inputs, trace=False):
    from concourse.bass_utils import run_bass_kernel_spmd

    st = prep_structure(np.asarray(inputs["edge_index"]), N_NODES, N_CORES)
    key = (st["SH"], st["TT"], st["NMM"])
    if key not in _CACHE:
        _CACHE[key] = build_program(st, N_CORES)
    nc = _CACHE[key]
    in_maps = make_in_maps(inputs, st)
    res = run_bass_kernel_spmd(nc, in_maps, core_ids=list(range(N_CORES)),
                               trace=trace)
    outs = [np.asarray(res.results[c]["out"])[:st["shard"]] for c in range(N_CORES)]
    return np.concatenate(outs, axis=0).astype(np.float32), res


def kernel(**inputs):
    out, _ = run(inputs, trace=False)
    return out



# revision 26
# speedup vs baseline: 1.7594x; 1.0849x over previous
"""Trainium2 Bass kernel for DirectedNetworkFeatureExtractor (GAT+FC GNN), v2.

Structure (per layer, nodes sharded 8-way, transposed-state free):
  node phase: h = x @ Wg (PSUM), als/ald reductions, h -> h_bounce DRAM
  AllGather h table (bf16, 8 x 1568-row chunks, Shared out)
  edge phase per (window-group, chunk) section:
    dma_gather h[src] (256B rows, int16 idx, 4 table chunks)
    als per edge (G * a_s reduce), ald per edge via host-built one-hot S0T
    matmuls, p = exp(leaky_relu(als+ald)), segment-sum via host-built S0
    one-hot matmuls accumulating (sum p*h, sum p) per dst window in PSUM
  x2 = agg/sum_p + bias, relu; x1 = fc branch; concat -> next layer state.

v2 vs v1: S0/S0T one-hots precomputed on host (DRAM inputs, DMA-streamed),
window-impure gather tiles (fewer padded slots), AGC=1568, Shared AG out.
"""
import math
import sys

sys.path.insert(0, "/opt/trn_rl_repo")

import numpy as np
import ml_dtypes

import concourse.bass as bass
import concourse.bacc as bacc
import concourse.tile as tile
from concourse import mybir

BF = ml_dtypes.bfloat16
P = 128

N_NODES = 100_000
N_CORES = 8
HEADS = 4
NCHUNK = 4
GB = 4          # windows per gather group
AGC = 3136      # AllGather chunk rows (= CHSZ/8 so one AG = one table chunk)


# --------------------------------------------------------------------------
# host-side graph preprocessing (untimed)
# --------------------------------------------------------------------------
def prep_structure(edge_index, n_nodes, n_cores):
    src = np.asarray(edge_index[0]).astype(np.int64)
    dst = np.asarray(edge_index[1]).astype(np.int64)
    shard = n_nodes // n_cores
    W = math.ceil(shard / P)
    SH = W * P
    TAB = n_cores * SH
    CHSZ = TAB // NCHUNK
    NE = len(src)

    core = dst // shard
    dloc = dst - core * shard
    win = dloc // P
    # AG-chunk-major table layout
    agc = AGC
    assert SH % agc == 0 and CHSZ % (agc * n_cores) == 0
    loc = src % shard
    rnk = src // shard
    a0 = loc // agc
    grow = (a0 * n_cores + rnk) * agc + (loc % agc)   # table row
    chk = grow // CHSZ
    rel = (grow - chk * CHSZ).astype(np.int64)

    NG = math.ceil(W / GB)
    g_of_w = np.arange(W) // GB

    # section (g, c) sizes pooled across windows, max over cores
    cnt = np.zeros((n_cores, NG, NCHUNK), np.int64)
    np.add.at(cnt, (core, g_of_w[win], chk), 1)
    sec_tiles = (cnt.max(axis=0) + P - 1) // P        # [NG, NCHUNK]

    # global tile base per section; plan in (g, then c) order
    plan = []
    t_run = 0
    for g in range(NG):
        ws = list(range(g * GB, min((g + 1) * GB, W)))
        g_off = t_run
        secs = []
        for c in range(NCHUNK):
            nt = int(sec_tiles[g, c])
            secs.append(dict(off=t_run, ntiles=nt, mm=None))
            t_run += nt
        plan.append(dict(windows=ws, off=g_off,
                         ntiles=t_run - g_off, secs=secs))
    TT = t_run

    idx16 = np.zeros((n_cores, TT * P), np.int16)

    # per-core slot assignment: edges sorted by (group, chunk, window, dloc)
    order = np.lexsort((np.arange(NE), dloc, chk, g_of_w[win], core))
    s_core = core[order]
    s_chk = chk[order]
    s_g = g_of_w[win[order]]
    s_win = win[order]
    s_rel = rel[order]
    s_dloc = dloc[order]

    # position within each (core, g, c) run
    gid = (s_core * NG + s_g) * NCHUNK + s_chk
    first = np.r_[True, gid[1:] != gid[:-1]]
    starts = np.flatnonzero(first)
    run_id = np.cumsum(first) - 1
    pos = np.arange(NE) - starts[run_id]

    sec_base = np.zeros((NG, NCHUNK), np.int64)
    for g in range(NG):
        for c in range(NCHUNK):
            sec_base[g, c] = plan[g]["secs"][c]["off"]
    tile_i = sec_base[s_g, s_chk] + pos // P
    part_i = pos % P
    idx16[s_core, tile_i * P + part_i] = s_rel.astype(np.int16)

    # slot -> (window, local slot) per core; -1 window means pad
    slot_win = np.full((n_cores, TT * P), -1, np.int8)   # window - g*GB
    slot_d = np.zeros((n_cores, TT * P), np.int8)        # dloc % 128
    slot_win[s_core, tile_i * P + part_i] = (s_win - s_g * GB).astype(np.int8)
    slot_d[s_core, tile_i * P + part_i] = (s_dloc % P).astype(np.int8)

    # ---- MM entries per section: (tile, local window) pairs with edges.
    # Shared across cores: entry exists if ANY core has edges there; S0/S0T
    # content is per-core.  Entry order: per section, by (window, tile).
    entries = []           # list of (g, c, tile_global, wl)
    for g in range(NG):
        nw = len(plan[g]["windows"])
        for ci in range(NCHUNK):
            sec = plan[g]["secs"][ci]
            nt = sec["ntiles"]
            t0 = sec["off"]
            if nt == 0:
                sec["mm"] = []
                sec["mm0"] = len(entries)
                continue
            # which (tile, wl) pairs are nonempty on any core
            sw = slot_win[:, t0 * P:(t0 + nt) * P].reshape(n_cores, nt, P)
            mm = []
            for wl in range(nw):
                tiles = np.flatnonzero((sw == wl).any(axis=(0, 2)))
                for t in tiles:
                    mm.append((int(t), wl))
            # order by tile then window for PSUM run grouping by window:
            # group by window (runs), tiles ascending within run
            mm.sort(key=lambda x: (x[1], x[0]))
            sec["mm"] = mm
            sec["mm0"] = len(entries)
            for (t, wl) in mm:
                entries.append((g, ci, t0 + t, wl))
    NMM = len(entries)

    # host one-hots per core: S0 [NMM, P(e), P(d)], S0T transposed
    S0 = np.zeros((n_cores, NMM, P, P), BF)
    e_idx = np.arange(P)
    for mi, (g, ci, tg, wl) in enumerate(entries):
        for cr in range(n_cores):
            swv = slot_win[cr, tg * P:(tg + 1) * P]
            sdv = slot_d[cr, tg * P:(tg + 1) * P]
            m = swv == wl
            if m.any():
                S0[cr, mi, e_idx[m], sdv[m].astype(np.int64)] = 1.0
    # pack idx for dma_gather: element i at [r, i//16] for r%16 == i%16
    j = np.arange(TT * 8)
    r16 = np.arange(16)
    packed = idx16[:, (j[None, :] * 16 + r16[:, None]).reshape(16, -1)]
    idx_packed = np.tile(packed, (1, 8, 1))       # [cores, 128, TT*8]

    # partition-major one-hot streams: S0pm[c][e, mi*P+d], S0Tpm[c][d, mi*P+e]
    S0pm = np.ascontiguousarray(
        np.transpose(S0, (0, 2, 1, 3)).reshape(n_cores, P, NMM * P))
    S0Tpm = np.ascontiguousarray(
        np.transpose(S0, (0, 3, 1, 2)).reshape(n_cores, P, NMM * P))

    return dict(
        shard=shard, W=W, SH=SH, TT=TT, TAB=TAB, CHSZ=CHSZ, NG=NG,
        plan=plan, NMM=NMM, agc=agc,
        idx=np.ascontiguousarray(idx_packed),
        S0=S0pm,
        S0T=S0Tpm,
    )


def prep_weights(inputs):
    def blocks(w):
        k = w.shape[0]
        return np.ascontiguousarray(w.reshape(k // P, P, w.shape[1]).astype(BF))

    def rep_row(v):
        return np.broadcast_to(np.asarray(v, np.float32), (P, P)).copy()

    g = lambda n: np.asarray(inputs[n], np.float32)
    layers = [dict(
        gw=blocks(g("g1_W")), fw=blocks(g("fc1_W")),
        a_s=rep_row(g("g1_as").reshape(-1)).astype(BF),
        a_d=rep_row(g("g1_ad").reshape(-1)).astype(BF),
        gb=rep_row(g("g1_b")), fb=g("fc1_b").reshape(P, 1).astype(np.float32),
    )]
    for i in range(2):
        layers.append(dict(
            gw=blocks(g("mg_W")[i]), fw=blocks(g("mfc_W")[i]),
            a_s=rep_row(g("mg_as")[i].reshape(-1)).astype(BF),
            a_d=rep_row(g("mg_ad")[i].reshape(-1)).astype(BF),
            gb=rep_row(g("mg_b")[i]), fb=g("mfc_b")[i].reshape(P, 1).astype(np.float32),
        ))
    layers.append(dict(
        gw=blocks(g("fg_W")), fw=blocks(g("ffc_W")),
        a_s=rep_row(g("fg_as").reshape(-1)).astype(BF),
        a_d=rep_row(g("fg_ad").reshape(-1)).astype(BF),
        gb=rep_row(g("fg_b")), fb=rep_row(g("ffc_b")),
    ))
    return layers


# --------------------------------------------------------------------------
# device program
# --------------------------------------------------------------------------
def build_program(st, n_cores):
    SH, W, TT, TAB, CHSZ = st["SH"], st["W"], st["TT"], st["TAB"], st["CHSZ"]
    agc = st["agc"]
    plan = st["plan"]
    NMM = st["NMM"]
    dt = mybir.dt
    f32, bf16, i16, i8 = dt.float32, dt.bfloat16, dt.int16, dt.int8
    HL = [HEADS, HEADS, HEADS, 1]
    GNTMAX = max(g["ntiles"] for g in plan)
    SECMAX = max(s["ntiles"] for g in plan for s in g["secs"])
    MMMAX = max(len(s["mm"]) for g in plan for s in g["secs"])

    nc = bacc.Bacc(None, num_swdge_queues=4)

    def inp(name, shape, d):
        return nc.declare_dram_parameter(name, list(shape), d, isOutput=False)

    x_in = inp("x", (SH, P), f32)
    idx_in = inp("idx", (P, TT * 8), i16)
    s0_in = inp("S0", (P, NMM * P), bf16)
    s0t_in = inp("S0T", (P, NMM * P), bf16)
    iota_in = inp("iota8", (P, P), i8)
    iotac_in = inp("iotac8", (P, 1), i8)
    lw = []
    for L in range(4):
        K = 1 if L == 0 else 2
        lw.append(dict(
            gw=inp(f"gw{L}", (K, P, P), bf16),
            fw=inp(f"fw{L}", (K, P, P), bf16),
            a_s=inp(f"as{L}", (P, P), bf16),
            a_d=inp(f"ad{L}", (P, P), bf16),
            gb=inp(f"gb{L}", (P, P), f32),
            fb=inp(f"fb{L}", (P, 1) if L < 3 else (P, P), f32),
        ))
    out_t = nc.declare_dram_parameter("out", [SH, P], f32, isOutput=True)

    CH = 512
    chunks = [(c, min(CH, SH - c)) for c in range(0, SH, CH)]

    with tile.TileContext(nc) as tc:
        with (
            tc.tile_pool(name="res", bufs=1) as res,
            tc.tile_pool(name="wts", bufs=1) as wts,
            tc.tile_pool(name="nwork", bufs=3) as nwork,
            tc.tile_pool(name="ework", bufs=3) as ework,
            tc.tile_pool(name="psA", bufs=2, space="PSUM") as psA,
            tc.tile_pool(name="psB", bufs=2, space="PSUM") as psB,
            tc.tile_pool(name="dram", bufs=1, space="DRAM") as dram,
        ):
            # ---------------- residents
            iota8 = res.tile([P, P], i8)
            nc.sync.dma_start(iota8[:], iota_in[:])
            iotac8 = res.tile([P, 1], i8)
            nc.sync.dma_start(iotac8[:], iotac_in[:])
            ident = res.tile([P, P], bf16)
            nc.vector.tensor_tensor(
                out=ident[:], in0=iotac8[:].to_broadcast([P, P]), in1=iota8[:],
                op=mybir.AluOpType.is_equal)

            wt = []
            for L in range(4):
                K = 1 if L == 0 else 2
                d = {}
                for nm in ("gw", "fw"):
                    t_ = wts.tile([P, K, P], bf16, tag=f"{nm}{L}")
                    nc.sync.dma_start(t_[:], lw[L][nm][:].rearrange("k p q -> p k q"))
                    d[nm] = t_
                for nm in ("a_s", "a_d"):
                    t_ = wts.tile([P, P], bf16, tag=f"{nm}{L}")
                    nc.sync.dma_start(t_[:], lw[L][nm][:])
                    d[nm] = t_
                t_ = wts.tile([P, P], f32, tag=f"gb{L}")
                nc.sync.dma_start(t_[:], lw[L]["gb"][:])
                d["gb"] = t_
                t_ = wts.tile([P, 1] if L < 3 else [P, P], f32, tag=f"fb{L}")
                nc.sync.dma_start(t_[:], lw[L]["fb"][:])
                d["fb"] = t_
                wt.append(d)

            # ---------------- DRAM scratch
            sA = [dram.tile([P, SH], bf16, tag=f"sA{i}", name=f"sA{i}") for i in range(3)]
            sB = [dram.tile([P, SH], bf16, tag=f"sB{i}", name=f"sB{i}") for i in range(3)]
            h_bounce = dram.tile([SH, P], bf16, tag="hb")
            tabs = [[dram.tile([CHSZ, P], bf16, tag=f"tab{L}_{ci}",
                               name=f"tab{L}_{ci}", addr_space="Shared")
                     for ci in range(NCHUNK)] for L in range(4)]
            x4_dram = dram.tile([SH, P], bf16, tag="x4")

            # ---------------- x -> transposed state
            for i in range(W):
                xt = nwork.tile([P, P], f32, tag="xin")
                nc.sync.dma_start(xt[:], x_in[i * P:(i + 1) * P, :])
                xb = nwork.tile([P, P], bf16, tag="xbf")
                nc.vector.tensor_copy(out=xb[:], in_=xt[:])
                tp = psB.tile([P, P], bf16, tag="tp", bufs=1)
                nc.tensor.transpose(out=tp[:], in_=xb[:], identity=ident[:])
                xTb = nwork.tile([P, P], bf16, tag="xT")
                nc.vector.tensor_copy(out=xTb[:], in_=tp[:])
                nc.sync.dma_start(sA[0][:, i * P:(i + 1) * P], xTb[:])

            # ---------------- layers
            for L in range(4):
                K = 1 if L == 0 else 2
                H = HL[L]
                C = P // H
                w = wt[L]
                if L == 0:
                    in_blk = [sA[0]]
                elif L == 1:
                    in_blk = [sA[1], sB[1]]
                elif L == 2:
                    for bs1, bs2, bd in ((sA[1], sA[2], sA[0]), (sB[1], sB[2], sB[0])):
                        for c0, cl in chunks:
                            a_ = nwork.tile([P, CH], bf16, tag="resid_a")
                            nc.sync.dma_start(a_[:, :cl], bs1[:, c0:c0 + cl])
                            b_ = nwork.tile([P, CH], bf16, tag="resid_b")
                            nc.sync.dma_start(b_[:, :cl], bs2[:, c0:c0 + cl])
                            nc.vector.tensor_add(out=a_[:, :cl], in0=a_[:, :cl], in1=b_[:, :cl])
                            nc.sync.dma_start(bd[:, c0:c0 + cl], a_[:, :cl])
                    in_blk = [sA[0], sB[0]]
                else:
                    in_blk = [sA[1], sB[1]]

                # ---- node phase
                ald_sb = res.tile([P, W, 4], f32, tag=f"aldsb{L % 2}")
                if H < 4:
                    nc.vector.memset(ald_sb[:], 0.0)
                for i in range(W):
                    inT = []
                    for k in range(K):
                        it = nwork.tile([P, P], bf16, tag=f"inT{k}")
                        nc.sync.dma_start(it[:], in_blk[k][:, i * P:(i + 1) * P])
                        inT.append(it)
                    hp = psA.tile([P, P], f32, tag="hp")
                    for k in range(K):
                        nc.tensor.matmul(out=hp[:], lhsT=inT[k][:], rhs=w["gw"][:, k, :],
                                         start=(k == 0), stop=(k == K - 1))
                    hb = nwork.tile([P, P], bf16, tag="hbf")
                    nc.vector.tensor_copy(out=hb[:], in_=hp[:])
                    nc.sync.dma_start(h_bounce[i * P:(i + 1) * P, :], hb[:])
                    tm = nwork.tile([P, P], f32, tag="adtmp")
                    nc.vector.tensor_tensor(out=tm[:], in0=hb[:], in1=w["a_d"][:],
                                            op=mybir.AluOpType.mult)
                    nc.vector.reduce_sum(
                        out=ald_sb[:, i, 0:H],
                        in_=tm[:].rearrange("p (h c) -> p h c", h=H),
                        axis=mybir.AxisListType.X)
                    if L == 3:
                        xp = psB.tile([P, CH], f32, tag="x1p")
                        for k in range(K):
                            nc.tensor.matmul(out=xp[:, :P], lhsT=inT[k][:],
                                             rhs=w["fw"][:, k, :],
                                             start=(k == 0), stop=(k == K - 1))
                        x4t = nwork.tile([P, P], f32, tag="x4t")
                        nc.vector.tensor_add(out=x4t[:], in0=xp[:, :P], in1=w["fb"][:])
                        nc.vector.tensor_scalar_max(out=x4t[:], in0=x4t[:], scalar1=0.0)
                        x4b = nwork.tile([P, P], bf16, tag="x4b")
                        nc.vector.tensor_copy(out=x4b[:], in_=x4t[:])
                        nc.sync.dma_start(x4_dram[i * P:(i + 1) * P, :], x4b[:])
                ald_bf = res.tile([P, W * 4], bf16, tag=f"aldbf{L % 2}")
                nc.vector.tensor_copy(out=ald_bf[:], in_=ald_sb[:].rearrange("p w c -> p (w c)"))

                # ---- AllGather h, one call per table chunk (Shared outs)
                for ci in range(NCHUNK):
                    nc.gpsimd.collective_compute(
                        "AllGather", mybir.AluOpType.bypass,
                        replica_groups=[list(range(n_cores))],
                        ins=[h_bounce[ci * agc:(ci + 1) * agc, :]],
                        outs=[tabs[L][ci][:]],
                    )

                # ---- x1 phase (overlaps AG)
                if L < 3:
                    outA = [sA[1], sA[2], sA[1]][L]
                    for c0, cl in chunks:
                        acc = psB.tile([P, CH], f32, tag="x1p")
                        rhs = []
                        for k in range(K):
                            it = nwork.tile([P, CH], bf16, tag=f"x1in{k}")
                            nc.sync.dma_start(it[:, :cl], in_blk[k][:, c0:c0 + cl])
                            rhs.append(it)
                        for k in range(K):
                            nc.tensor.matmul(out=acc[:, :cl], lhsT=w["fw"][:, k, :],
                                             rhs=rhs[k][:, :cl],
                                             start=(k == 0), stop=(k == K - 1))
                        x1b = nwork.tile([P, CH], bf16, tag="x1b")
                        nc.scalar.activation(out=x1b[:, :cl], in_=acc[:, :cl],
                                             func=mybir.ActivationFunctionType.Relu,
                                             bias=w["fb"][:], scale=1.0)
                        nc.sync.dma_start(outA[:, c0:c0 + cl], x1b[:, :cl])

                # ---- edge phase
                for g in plan:
                    gt0, gnt = g["off"], g["ntiles"]
                    nw = len(g["windows"])
                    idxg = ework.tile([P, GNTMAX * 8], i16, tag="idxg")
                    nc.sync.dma_start(idxg[:, :gnt * 8], idx_in[:, gt0 * 8:(gt0 + gnt) * 8])
                    G = ework.tile([P, GNTMAX, P], bf16, tag="G")
                    for ci in range(NCHUNK):
                        sec = g["secs"][ci]
                        nt = sec["ntiles"]
                        if nt == 0:
                            continue
                        sl = sec["off"] - gt0
                        nc.gpsimd.dma_gather(
                            G[:, sl:sl + nt, :],
                            tabs[L][ci][:],
                            idxg[:, sl * 8:(sl + nt) * 8],
                            num_idxs=nt * P, num_idxs_reg=nt * P, elem_size=P,
                            single_packet=False, queue_num=ci)

                    als = ework.tile([P, GNTMAX * 4], bf16, tag="als")
                    lg = ework.tile([P, GNTMAX * 4], f32, tag="lg")
                    for ci in range(NCHUNK):
                        sec = g["secs"][ci]
                        nt = sec["ntiles"]
                        if nt == 0:
                            continue
                        sl = sec["off"] - gt0
                        nmm = len(sec["mm"])
                        # S0T for this section (partition-major: [d, m, e])
                        s0t_sb = ework.tile([P, MMMAX, P], bf16, tag="s0t")
                        nc.scalar.dma_start(
                            s0t_sb[:, :nmm, :],
                            s0t_in[:, sec["mm0"] * P:(sec["mm0"] + nmm) * P]
                            .rearrange("d (m e) -> d m e", m=nmm))
                        tmp = ework.tile([P, SECMAX, P], bf16, tag="tmp")
                        nc.vector.tensor_tensor(
                            out=tmp[:, :nt, :], in0=G[:, sl:sl + nt, :],
                            in1=w["a_s"][:].rearrange("p q -> p () q").to_broadcast([P, nt, P]),
                            op=mybir.AluOpType.mult)
                        with nc.allow_low_precision(reason="als logits tolerate bf16"):
                            nc.vector.reduce_sum(
                                out=als[:, sl * H:(sl + nt) * H],
                                in_=tmp[:, :nt, :].rearrange("p t (h c) -> p (t h) c", h=H),
                                axis=mybir.AxisListType.X)
                        # ald per edge: accumulate over mm entries per tile
                        aldp = psA.tile([P, SECMAX, 4], f32, tag="ald", bufs=1)
                        tile_first = {}
                        tile_last = {}
                        for mi, (t, wl) in enumerate(sec["mm"]):
                            tile_first.setdefault(t, mi)
                            tile_last[t] = mi
                        for mi, (t, wl) in enumerate(sec["mm"]):
                            wi = g["windows"][wl]
                            nc.tensor.matmul(
                                out=aldp[:, t, 0:H],
                                lhsT=s0t_sb[:, mi, :],
                                rhs=ald_bf[:, wi * 4:wi * 4 + H],
                                start=(tile_first[t] == mi),
                                stop=(tile_last[t] == mi))
                        nc.vector.tensor_add(
                            out=lg[:, sl * H:(sl + nt) * H],
                            in0=als[:, sl * H:(sl + nt) * H],
                            in1=aldp[:, :nt, 0:H].rearrange("p t h -> p (t h)"))

                    lr = ework.tile([P, GNTMAX * 4], f32, tag="lr")
                    nc.vector.scalar_tensor_tensor(
                        out=lr[:, :gnt * H], in0=lg[:, :gnt * H], scalar=0.2,
                        in1=lg[:, :gnt * H],
                        op0=mybir.AluOpType.mult, op1=mybir.AluOpType.max)
                    pe_t = ework.tile([P, GNTMAX * 4], bf16, tag="pe")
                    nc.scalar.activation(out=pe_t[:, :gnt * H], in_=lr[:, :gnt * H],
                                         func=mybir.ActivationFunctionType.Exp)

                    x2acc = ework.tile([P, GB, P + 4], f32, tag="x2acc")
                    wdone = {}
                    for ci in range(NCHUNK):
                        sec = g["secs"][ci]
                        nt = sec["ntiles"]
                        if nt == 0:
                            continue
                        sl = sec["off"] - gt0
                        nmm = len(sec["mm"])
                        s0_sb = ework.tile([P, MMMAX, P], bf16, tag="s0")
                        nc.scalar.dma_start(
                            s0_sb[:, :nmm, :],
                            s0_in[:, sec["mm0"] * P:(sec["mm0"] + nmm) * P]
                            .rearrange("e (m d) -> e m d", m=nmm))
                        GW = ework.tile([P, SECMAX, P + 4], bf16, tag="GW")
                        nc.vector.tensor_tensor(
                            out=GW[:, :nt, 0:P].rearrange("p t (h c) -> p t h c", h=H),
                            in0=G[:, sl:sl + nt, :].rearrange("p t (h c) -> p t h c", h=H),
                            in1=pe_t[:, sl * H:(sl + nt) * H]
                                .rearrange("p (t h) -> p t h ()", h=H)
                                .to_broadcast([P, nt, H, C]),
                            op=mybir.AluOpType.mult)
                        nc.vector.tensor_copy(
                            out=GW[:, :nt, P:P + H],
                            in_=pe_t[:, sl * H:(sl + nt) * H].rearrange("p (t h) -> p t h", h=H))
                        # aggregate per window: entries sorted by (wl, tile)
                        mm = sec["mm"]
                        mi = 0
                        while mi < len(mm):
                            wl = mm[mi][1]
                            mj = mi
                            while mj < len(mm) and mm[mj][1] == wl:
                                mj += 1
                            aggp = psA.tile([P, P + 4], f32, tag="agg")
                            for k in range(mi, mj):
                                t = mm[k][0]
                                nc.tensor.matmul(
                                    out=aggp[:, :P + H],
                                    lhsT=s0_sb[:, k, :],
                                    rhs=GW[:, t, 0:P + H],
                                    start=(k == mi), stop=(k == mj - 1))
                            wi = g["windows"][wl]
                            if wi not in wdone:
                                wdone[wi] = True
                                nc.vector.tensor_copy(
                                    out=x2acc[:, wl, 0:P + H], in_=aggp[:, :P + H])
                            else:
                                nc.vector.tensor_add(
                                    out=x2acc[:, wl, 0:P + H],
                                    in0=x2acc[:, wl, 0:P + H], in1=aggp[:, :P + H])
                            mi = mj

                    # batched window finish for the whole group
                    nw = len(g["windows"])
                    wi0 = g["windows"][0]
                    sinv = ework.tile([P, GB, 4], f32, tag="sinv")
                    nc.vector.tensor_scalar_add(
                        out=sinv[:, :nw, 0:H], in0=x2acc[:, :nw, P:P + H],
                        scalar1=1e-16)
                    nc.vector.reciprocal(out=sinv[:, :nw, 0:H],
                                         in_=sinv[:, :nw, 0:H])
                    x2g = ework.tile([P, GB, P], f32, tag="x2g")
                    nc.vector.tensor_tensor(
                        out=x2g[:, :nw, :].rearrange("p w (h c) -> p w h c", h=H),
                        in0=x2acc[:, :nw, 0:P].rearrange("p w (h c) -> p w h c", h=H),
                        in1=sinv[:, :nw, 0:H].rearrange("p w h -> p w h ()")
                            .to_broadcast([P, nw, H, C]),
                        op=mybir.AluOpType.mult)
                    nc.vector.tensor_add(
                        out=x2g[:, :nw, :], in0=x2g[:, :nw, :],
                        in1=w["gb"][:].rearrange("p q -> p () q")
                            .to_broadcast([P, nw, P]))
                    nc.vector.tensor_scalar_max(
                        out=x2g[:, :nw, :], in0=x2g[:, :nw, :], scalar1=0.0)
                    if L < 3:
                        x2b = ework.tile([P, GB, P], bf16, tag="x2b")
                        nc.vector.tensor_copy(out=x2b[:, :nw, :], in_=x2g[:, :nw, :])
                        outB = [sB[1], sB[2], sB[1]][L]
                        for wl, wi in enumerate(g["windows"]):
                            tp = psB.tile([P, P], bf16, tag="tp", bufs=1)
                            nc.tensor.transpose(out=tp[:], in_=x2b[:, wl, :],
                                                identity=ident[:])
                            x2T = ework.tile([P, P], bf16, tag="x2T")
                            nc.vector.tensor_copy(out=x2T[:], in_=tp[:])
                            nc.sync.dma_start(outB[:, wi * P:(wi + 1) * P], x2T[:])
                    else:
                        x4t = ework.tile([P, GB, P], bf16, tag="x4in")
                        nc.sync.dma_start(
                            x4t[:, :nw, :],
                            x4_dram[wi0 * P:(wi0 + nw) * P, :]
                            .rearrange("(w p) q -> p w q", p=P))
                        yo = ework.tile([P, GB, P], f32, tag="yo")
                        nc.vector.tensor_add(out=yo[:, :nw, :],
                                             in0=x2g[:, :nw, :], in1=x4t[:, :nw, :])
                        nc.sync.dma_start(
                            out_t[wi0 * P:(wi0 + nw) * P, :]
                            .rearrange("(w p) q -> p w q", p=P),
                            yo[:, :nw, :])

    nc.compile()
    return nc


# --------------------------------------------------------------------------
# runner
# --------------------------------------------------------------------------
def make_in_maps(inputs, st):
    x = np.asarray(inputs["x"], np.float32)
    shard, SH = st["shard"], st["SH"]
    layers = prep_weights(inputs)
    iota8 = np.broadcast_to(np.arange(P, dtype=np.int8), (P, P)).copy()
    iotac8 = np.arange(P, dtype=np.int8).reshape(P, 1).copy()

    common = {"iota8": iota8, "iotac8": iotac8}
    for L, lwd in enumerate(layers):
        common[f"gw{L}"] = lwd["gw"]
        common[f"fw{L}"] = lwd["fw"]
        common[f"as{L}"] = lwd["a_s"]
        common[f"ad{L}"] = lwd["a_d"]
        common[f"gb{L}"] = lwd["gb"]
        common[f"fb{L}"] = lwd["fb"]

    in_maps = []
    for c in range(N_CORES):
        xs = np.zeros((SH, P), np.float32)
        xs[:shard] = x[c * shard:(c + 1) * shard]
        m = dict(common)
        m["x"] = xs
        m["idx"] = st["idx"][c]
        m["S0"] = st["S0"][c]
        m["S0T"] = st["S0T"][c]
        in_maps.append(m)
    return in_maps


_CACHE = {}


def run(inputs, trace=False):
    from concourse.bass_utils import run_bass_kernel_spmd

    st = prep_structure(np.asarray(inputs["edge_index"]), N_NODES, N_CORES)
    key = (st["SH"], st["TT"], st["NMM"])
    if key not in _CACHE:
        _CACHE[key] = build_program(st, N_CORES)
    nc = _CACHE[key]
    in_maps = make_in_maps(inputs, st)
    res = run_bass_kernel_spmd(nc, in_maps, core_ids=list(range(N_CORES)),
                               trace=trace)
    outs = [np.asarray(res.results[c]["out"])[:st["shard"]] for c in range(N_CORES)]
    return np.concatenate(outs, axis=0).astype(np.float32), res


def kernel(**inputs):
    out, _ = run(inputs, trace=False)
    return out

